# revision 1
# baseline (speedup 1.0000x reference)
"""Trainium2 Bass kernel for nn_BinaryTokenClassificationModel (segment_reduce).

Math: logits[b,i,j] = dot(segmean(1+i), w_src) + dot(segmean(513+j), w_tgt) + bias,
where segmean(s) is the mean of outputs[b] over the s-th consecutive run of equal
word_ids (attention_mask is all ones here).  dot commutes with the segment mean,
so per-token dots v[t,c] = x[t].w_c suffice; segment sums of v are accumulated by
PE one-hot matmuls and scaled by host-computed 1/count at the very end.

Design (final, DMA-roofline oriented):
  - Only tokens of segments 1..1024 are staged (host gathers them REVERSED, so
    tgt segments 1024..513 stream first, then src 512..1).  NT = ceil(max/128)
    tiles of 128 tokens; short examples padded with slo=-1 dummies.
  - x is cast f32->bf16 during the SWDGE (gpsimd) DMA: HBM reads stay f32 (the
    mandatory roofline) but on-chip compute runs at 16-bit rates.  All x DMAs
    are issued up-front in 8 chunks so the HBM stream is continuous.
  - Per tile: DVE tensor_tensor multiplies x by the replicated weight row (bf16
    2x mode, ~690ns); the h-reduction to v is split between ACT's fused
    activation-accumulate and DVE tensor_reduce so neither engine exceeds the
    DMA stream time.  gpsimd builds the tiny per-u mask r_t = ch*v (bf16), and
    PE accumulates pool[s_lo, u] += onehot(s_lo)^T @ r_t with all-bf16 matmuls
    into small PSUM regions (tgt: u 4..8, src: u 0..4, late-src: u 0).
  - Counts never touch the device loop: host bakes 1/count into tiny [128,5]
    tables applied at the tail.  The tgt half of the output (broadcast row) and
    blocks 1-3 are emitted early, hidden under the src-phase DMA stream; only
    block 0 (which needs the last tiles) remains in the tail.
  - Output is written bf16 (tolerance 2e-2; bf16 error ~5e-3) and upcast on host.

Sharding: pure data parallel, one example (B=8) per NeuronCore (8 cores).
"""
import sys

for _p in ("/opt/trn_rl_repo", "/root/.axon_site/_ro/trn_rl_repo"):
    if _p not in sys.path:
        sys.path.append(_p)

from contextlib import ExitStack

import ml_dtypes
import numpy as np

import concourse.bacc as bacc
import concourse.bass as bass
import concourse.tile as tile
from concourse import mybir
from concourse.bass_utils import run_bass_kernel_spmd

F32 = mybir.dt.float32
BF16 = mybir.dt.bfloat16
P = 128
H = 1024
AL = mybir.AluOpType
ACTF = mybir.ActivationFunctionType

# x-tile DMA chunking (tiles per SWDGE dma_start); first chunk small so
# compute starts early, last chunks small to keep the post-stream tail short
def _chunks_for(NT):
    sizes = []
    rem = NT
    plan = [1, 2, 2, 2, 2, 2, 2, 2]
    for s in plan:
        if rem <= 4:
            break
        k = min(s, rem - 4)
        sizes.append(k)
        rem -= k
    sizes += [1] * rem
    starts = np.cumsum([0] + sizes[:-1]).tolist()
    return list(zip(starts, sizes))


def _build_nc(NT: int, ops: list, CW: int, lt_tgt: int, lt_s1: int, ct: dict) -> bass.Bass:
    NCT = max(ct["NCT"], 1)
    NCF = CW + 11               # ch_all | rcnt_src | rcnt_tgt | bias
    NCB = 4 * P + 12            # s1 | s2 | ident | iota | zeros(12)
    nc = bacc.Bacc("TRN2", target_bir_lowering=False, debug=False, num_devices=8)
    x_d = nc.declare_dram_parameter("x", [NT * P, H], F32, isOutput=False)
    cf_d = nc.declare_dram_parameter("consts", [P, NCF], F32, isOutput=False)
    cb_d = nc.declare_dram_parameter("cbf", [P, NCB], BF16, isOutput=False)
    cl_d = nc.declare_dram_parameter("clsb", [P, (NT + NCT) * P], BF16, isOutput=False)
    wb_d = nc.declare_dram_parameter("wrepb", [P, 2 * H], BF16, isOutput=False)
    y_d = nc.declare_dram_parameter("y", [512, 512], BF16, isOutput=True)

    with tile.TileContext(nc) as tc, ExitStack() as ctx:
        consts = ctx.enter_context(tc.tile_pool(name="consts", bufs=1))
        clp = ctx.enter_context(tc.tile_pool(name="clp", bufs=1))
        xpool = ctx.enter_context(tc.tile_pool(name="xp", bufs=1))
        scrp = ctx.enter_context(tc.tile_pool(name="scr", bufs=6))
        scrp2 = ctx.enter_context(tc.tile_pool(name="scr2", bufs=4))
        vpool = ctx.enter_context(tc.tile_pool(name="vp", bufs=12))
        rpool = ctx.enter_context(tc.tile_pool(name="rp", bufs=8))
        segp = ctx.enter_context(tc.tile_pool(name="segp", bufs=1))
        opool = ctx.enter_context(tc.tile_pool(name="op", bufs=4))
        pp_pool = ctx.enter_context(tc.tile_pool(name="ppool", bufs=1, space="PSUM"))
        pp_cs = ctx.enter_context(tc.tile_pool(name="pcs", bufs=1, space="PSUM"))
        pp_row = ctx.enter_context(tc.tile_pool(name="prow", bufs=1, space="PSUM"))
        pp_ms = ctx.enter_context(tc.tile_pool(name="pms", bufs=1, space="PSUM"))

        # ---- HWDGE (sync) queue: weights + consts, later the output ----
        wrep = consts.tile([P, 2 * H], BF16)
        nc.sync.dma_start(out=wrep[:, H:2 * H], in_=wb_d[:, H:2 * H])  # w_tgt first
        cb = consts.tile([P, NCB], BF16)
        nc.sync.dma_start(out=cb, in_=cb_d[:])
        cf = consts.tile([P, NCF], F32)
        nc.sync.dma_start(out=cf, in_=cf_d[:])
        nc.sync.dma_start(out=wrep[:, 0:H], in_=wb_d[:, 0:H])

        ch_all = cf[:, 0:CW]
        rcS = cf[:, CW:CW + 5]
        rcT = cf[:, CW + 5:CW + 10]
        biascol = cf[:, CW + 10:CW + 11]
        s1 = cb[:, 0:P]
        s2 = cb[:, P:2 * P]
        ident = cb[:, 2 * P:3 * P]
        iota = cb[:, 3 * P:4 * P]
        zeros8 = cb[:, 4 * P:4 * P + 12]

        # ---- PSUM pools, zero-initialized via start=True matmuls ----
        # The ladder accumulates per-token dots into pool_t/pool_s; the C-tail
        # accumulates raw x segment-sums [s%128, h] into buf_u (u = 0, 1) and
        # dots them with w only at drain time.
        pool_ts = pp_pool.tile([P, 10], F32)
        pool_t = pool_ts[:, 0:5]  # tgt sums: col j = u-4, row = s%128
        pool_s = pool_ts[:, 5:10]  # src sums: col j = u,  row = s%128
        nc.tensor.matmul(pool_ts, lhsT=iota, rhs=zeros8[:, 0:10], start=True,
                         stop=False, skip_group_check=True)
        buf_par = {0: pp_cs.tile([P, H], F32, name="bufu0"),
                   1: pp_cs.tile([P, H], F32, name="bufu1")}

        # ---- x stream: all chunks up-front on the SWDGE (gpsimd) queue,
        # cast f32->bf16 in the DMA datapath ----
        chunks = _chunks_for(NT)
        x_tiles = [None] * NT
        x_chunk = {}  # chunk start tile -> whole-chunk SBUF tile
        for c, (st, k) in enumerate(chunks):
            xc = xpool.tile([P, k, H], BF16, name=f"xc{c}")
            nc.gpsimd.dma_start(
                out=xc, in_=x_d[P * st:P * (st + k), :].rearrange("(k p) h -> p k h", p=P))
            x_chunk[st] = xc
            for j in range(k):
                x_tiles[st + j] = xc[:, j, :]

        clsb = clp.tile([P, NT + NCT, P], BF16)
        cl_all = clsb[:, 0:NT, :]
        cl_ct = clsb[:, NT:NT + NCT, :]
        nc.sync.dma_start(out=clsb[:, 0:4, :],
                          in_=cl_d[:, 0:4 * P].rearrange("p (i q) -> p i q", q=P))
        nc.sync.dma_start(out=clsb[:, 4:NT + NCT, :],
                          in_=cl_d[:, 4 * P:].rearrange("p (i q) -> p i q", q=P))

        # ---- main loop over token tiles ----
        rowb_sb = segp.tile([P, 512], BF16)
        msrcm14 = segp.tile([P, 5], BF16)

        def emit_block(k, rhs1, rhs2):
            msps = pp_ms.tile([P, 1], F32, name="msps", bufs=2)
            nc.tensor.matmul(msps, lhsT=s1, rhs=rhs1, start=True, stop=False,
                             skip_group_check=True)
            nc.tensor.matmul(msps, lhsT=s2, rhs=rhs2, start=False, stop=True,
                             skip_group_check=True)
            msv = segp.tile([P, 1], F32, name=f"msv{k}")
            nc.vector.tensor_copy(out=msv, in_=msps)
            lg = opool.tile([P, 512], BF16, name=f"lg{k}")
            nc.vector.tensor_scalar(out=lg, in0=rowb_sb, scalar1=msv,
                                    scalar2=None, op0=AL.add)
            nc.sync.dma_start(out=y_d[P * k:P * (k + 1), :], in_=lg)

        # TT grouping: maximal runs of single-op same-channel tiles within one
        # chunk share one wide [P, k, H] multiply (amortizes per-op cost)
        group_at = {}   # first tile -> run length
        for st, k in chunks:
            j = 0
            while j < k:
                i0 = st + j
                r = 1
                if len(ops[i0]) == 1:
                    while (j + r < k and len(ops[st + j + r]) == 1
                           and ops[st + j + r][0]["c"] == ops[i0][0]["c"]):
                        r += 1
                if r > 1:
                    group_at[i0] = (r, st, j)
                j += r

        # Two-phase emission per tile, offset by one: tile i's multiply is
        # issued BEFORE tile i-1's reduce/mask/matmul stage, so a DVE-side
        # reduce never blocks the next multiply that feeds ACT (software
        # pipelining of the DVE queue).
        scr_of = {}

        def emit_tt(i):
            if i in group_at:
                r, st, j = group_at[i]
                c01 = 1 if ops[i][0]["c"] == "tgt" else 0
                scr2 = scrp2.tile([P, 3, H], BF16, name="scr2")
                nc.vector.tensor_tensor(
                    out=scr2[:, 0:r, :], in0=x_chunk[st][:, j:j + r, :],
                    in1=wrep[:, c01 * H:(c01 + 1) * H].unsqueeze(1).to_broadcast((P, r, H)),
                    op=AL.mult)
                for q in range(r):
                    scr_of[(i + q, 0)] = scr2[:, q, :]
            for oi, e in enumerate(ops[i]):
                if (i, oi) not in scr_of:
                    c01 = 1 if e["c"] == "tgt" else 0
                    scr = scrp.tile([P, H], BF16, name="scr1")
                    nc.vector.tensor_tensor(out=scr, in0=x_tiles[i],
                                            in1=wrep[:, c01 * H:(c01 + 1) * H], op=AL.mult)
                    scr_of[(i, oi)] = scr

        def emit_rest(i):
            for oi, e in enumerate(ops[i]):
                scr = scr_of[(i, oi)]
                v = vpool.tile([P, 1], F32)
                if e["red"] == "dve":
                    nc.vector.tensor_reduce(out=v, in_=scr, axis=mybir.AxisListType.X,
                                            op=AL.add)
                else:
                    nc.scalar.activation(out=scr, in_=scr, func=ACTF.Copy, accum_out=v)
                nU = len(e["ulist"])
                r_t = rpool.tile([P, nU], BF16)
                off = e["ch_off"]
                nc.gpsimd.tensor_tensor(out=r_t, in0=ch_all[:, off:off + nU],
                                        in1=v.to_broadcast((P, nU)), op=AL.mult)
                if e["c"] == "tgt":
                    pool, col_lo, stop = pool_t, e["ulist"][0] - 4, i == lt_tgt
                else:
                    pool, col_lo, stop = pool_s, e["ulist"][0], i == lt_s1
                nc.tensor.matmul(pool[:, col_lo:col_lo + nU], lhsT=cl_all[:, i, :],
                                 rhs=r_t, start=False, stop=stop, skip_group_check=True)
            emit_late(i)

        def emit_late(i):
            if i == lt_tgt:
                # tgt tail early: broadcast row of the output, hidden under the
                # src-phase DMA stream
                mtgtm = segp.tile([P, 5], BF16)
                nc.vector.tensor_tensor(out=mtgtm, in0=pool_t, in1=rcT, op=AL.mult)
                rowb_ps = pp_row.tile([P, 512], F32)
                nc.tensor.matmul(rowb_ps[:, 0:127], lhsT=mtgtm[:, 0:1].to_broadcast((P, P)),
                                 rhs=ident[:, 1:128], start=True, stop=True)
                nc.tensor.matmul(rowb_ps[:, 127:255], lhsT=mtgtm[:, 1:2].to_broadcast((P, P)),
                                 rhs=ident, start=True, stop=True)
                nc.tensor.matmul(rowb_ps[:, 255:383], lhsT=mtgtm[:, 2:3].to_broadcast((P, P)),
                                 rhs=ident, start=True, stop=True)
                nc.tensor.matmul(rowb_ps[:, 383:511], lhsT=mtgtm[:, 3:4].to_broadcast((P, P)),
                                 rhs=ident, start=True, stop=True)
                nc.tensor.matmul(rowb_ps[:, 511:512], lhsT=mtgtm[:, 4:5].to_broadcast((P, P)),
                                 rhs=ident[:, 0:1], start=True, stop=True)
                nc.scalar.activation(out=rowb_sb, in_=rowb_ps, func=ACTF.Identity,
                                     bias=biascol, scale=1.0)
            if i == lt_s1:
                # pool_s closed: block 3 emitted under the x stream
                nc.vector.tensor_tensor(out=msrcm14, in0=pool_s, in1=rcS, op=AL.mult)
                emit_block(3, msrcm14[:, 3:4], msrcm14[:, 4:5])

        ct_start = ct["start"]
        emit_tt(0)
        for i in range(1, ct_start):
            emit_tt(i)
            emit_rest(i - 1)
        emit_rest(ct_start - 1)

        # ---- C-tail: PE accumulates raw x segment-sums per u; the w-dot
        # happens once per u at drain time ----
        vu1 = segp.tile([P, 1], F32)
        vu2 = segp.tile([P, 1], F32)
        scrd1 = segp.tile([P, H], BF16)
        va = segp.tile([P, 1], F32)
        vb = segp.tile([P, 1], F32)
        scrd = segp.tile([P, H], BF16)
        for i in range(ct_start, NT):
            for uv, tbl in ct["plan"][i]:
                lhsT = cl_all[:, i, :] if tbl[0] == "pri" else cl_ct[:, tbl[1], :]
                start = ct["first_u"][uv] == i
                stop = ct["last_u"][uv] == i
                for hh in (0, 1):
                    nc.tensor.matmul(buf_par[uv % 2][:, hh * 512:(hh + 1) * 512],
                                     lhsT=lhsT,
                                     rhs=x_tiles[i][:, hh * 512:(hh + 1) * 512],
                                     start=start, stop=stop, skip_group_check=True)
            if i == ct["last_u"].get(2):
                # drain u=2: dot with w_src; col 2 = ladder part + C part
                nc.vector.tensor_tensor(out=scrd, in0=buf_par[0], in1=wrep[:, 0:H],
                                        op=AL.mult)
                nc.scalar.activation(out=scrd, in_=scrd, func=ACTF.Copy, accum_out=vu2)
                nc.vector.tensor_scalar(out=msrcm14[:, 2:3], in0=pool_s[:, 2:3],
                                        scalar1=vu2, scalar2=rcS[:, 2:3],
                                        op0=AL.add, op1=AL.mult)
                emit_block(2, msrcm14[:, 2:3], msrcm14[:, 3:4])
            if i == ct["last_u"].get(1):
                # drain u=1 (C-only column), then block 1
                nc.vector.tensor_tensor(out=scrd1, in0=buf_par[1], in1=wrep[:, 0:H],
                                        op=AL.mult)
                nc.scalar.activation(out=scrd1, in_=scrd1, func=ACTF.Copy, accum_out=vu1)
                nc.vector.tensor_scalar(out=msrcm14[:, 1:2], in0=vu1, scalar1=0.0,
                                        scalar2=rcS[:, 1:2], op0=AL.add, op1=AL.mult)
                emit_block(1, msrcm14[:, 1:2], msrcm14[:, 2:3])

        # ---- final tail: drain u=0 split across DVE and ACT, then block 0 ----
        scra = segp.tile([P, 512], BF16)
        scrb = segp.tile([P, 512], BF16)
        nc.vector.tensor_tensor(out=scra, in0=buf_par[0][:, 0:512], in1=wrep[:, 0:512],
                                op=AL.mult)
        nc.vector.tensor_tensor(out=scrb, in0=buf_par[0][:, 512:1024],
                                in1=wrep[:, 512:1024], op=AL.mult)
        nc.scalar.activation(out=scra, in_=scra, func=ACTF.Copy, accum_out=va)
        nc.vector.tensor_reduce(out=vb, in_=scrb, axis=mybir.AxisListType.X, op=AL.add)
        nc.vector.tensor_scalar(out=msrcm14[:, 0:1], in0=va, scalar1=vb,
                                scalar2=rcS[:, 0:1], op0=AL.add, op1=AL.mult)
        emit_block(0, msrcm14[:, 0:1], msrcm14[:, 1:2])

    nc.compile()
    return nc


def _host_prep(inputs):
    x = np.asarray(inputs["outputs"], dtype=np.float32)
    wid = np.asarray(inputs["word_ids"]).astype(np.int64)
    cw = np.asarray(inputs["classifier_w"], dtype=np.float32)
    bias = np.float32(np.asarray(inputs["classifier_b"]))
    B, L, Hd = x.shape
    assert (Hd, L, B) == (H, 4096, 8)
    assert int(inputs["num_src"]) == 512 and int(inputs["num_tgt"]) == 512
    assert np.asarray(inputs["attention_mask"]).min() == 1

    segs, idxs = [], []
    for b in range(B):
        ns = np.ones(L, np.int64)
        ns[1:] = wid[b, 1:] != wid[b, :-1]
        seg = np.cumsum(ns) - 1
        keep = (seg >= 1) & (seg <= 1024)
        idxs.append(np.nonzero(keep)[0][::-1])  # descending segment order
        segs.append(seg)
    ntoks = [len(i) for i in idxs]
    NT = (max(ntoks) + P - 1) // P
    L2 = NT * P

    tok_s = np.full((B, L2), -1, np.int64)
    xbs = []
    for b in range(B):
        n = ntoks[b]
        tok_s[b, :n] = segs[b][idxs[b]]
        xi = np.zeros(L2, np.int64)
        xi[:n] = idxs[b]
        xbs.append(np.ascontiguousarray(x[b][xi]))

    is_t = tok_s >= 513
    is_s = (tok_s >= 1) & (tok_s <= 512)
    u = np.where(tok_s >= 0, tok_s >> 7, -1)
    slo_v = np.where(tok_s >= 0, tok_s & 127, -1)

    # program metadata, unioned over cores (same compiled program everywhere)
    ops = []
    for i in range(NT):
        sl = slice(i * P, (i + 1) * P)
        ent = []
        for cname, m in (("tgt", is_t), ("src", is_s)):
            msk = m[:, sl]
            if not msk.any():
                continue
            uu = u[:, sl][msk]
            ulist = list(range(int(uu.min()), int(uu.max()) + 1))
            assert len(ulist) <= 3
            ent.append(dict(c=cname, ulist=ulist, ch_off=None, red="act"))
        ops.append(ent)
    lt_tgt = max(i for i in range(NT) if any(e["c"] == "tgt" for e in ops[i]))
    # C-tail: trailing tiles whose src segments sit in u<=1 are handled by PE
    # segment-sum matmuls + two drains instead of the DVE/ACT ladder
    ct_start = NT
    while (ct_start > 0 and ops[ct_start - 1]
           and all(e["c"] == "src" and max(e["ulist"]) <= 2 for e in ops[ct_start - 1])):
        ct_start -= 1
    assert lt_tgt < ct_start < NT
    lt_s1 = ct_start - 1
    # ladder src tiles must never touch u<2 (their sums land in pool_s 2..4)
    for i in range(ct_start):
        for e in ops[i]:
            if e["c"] == "src":
                assert min(e["ulist"]) >= 2
    # C-tile plan: per tile a list of (u, table).  Single-u tiles use the
    # plain one-hot column; two-u tiles need parity-masked tables so u0/u1
    # sums stay in their own PSUM buffers.
    NCT = 0
    ct_cols = []  # (tile, u) per masked table column
    ct_plan = {}
    for i in range(ct_start, NT):
        e = ops[i][0]
        sides = []
        if len(e["ulist"]) == 1:
            sides.append((e["ulist"][0], ("pri",)))
        else:
            for uv in sorted(e["ulist"], reverse=True):
                sides.append((uv, ("ct", NCT)))
                ct_cols.append((i, uv))
                NCT += 1
        ct_plan[i] = sides
    first_u, last_u = {}, {}
    for i in range(ct_start, NT):
        for uv, _ in ct_plan[i]:
            first_u.setdefault(uv, i)
            last_u[uv] = i
    assert set(first_u) <= {0, 1, 2} and last_u.get(0) == NT - 1
    # ladder reduction split between ACT and DVE
    flat = [e for ent in ops[:ct_start] for e in ent]
    n = len(flat)
    for j in np.linspace(2, max(3, n - 2), 6).astype(int):
        if 0 <= j < n:
            flat[int(j)]["red"] = "dve"
    # ch mask columns for the ladder tiles
    CW, ch_cols = 0, []
    for i in range(ct_start):
        for e in ops[i]:
            e["ch_off"] = CW
            for uv in e["ulist"]:
                ch_cols.append((i, e["c"], uv))
            CW += len(e["ulist"])

    iota_h = np.broadcast_to(np.arange(P, dtype=np.float32), (P, P))
    s1_h = np.eye(P, k=-1, dtype=np.float32)  # s1[p,m]=1 iff m==p-1 -> out[m]=in[m+1]
    s2_h = np.zeros((P, P), np.float32)
    s2_h[0, P - 1] = 1.0
    ident_h = np.eye(P, dtype=np.float32)
    wrep_h = np.broadcast_to(cw, (P, 2 * H)).astype(ml_dtypes.bfloat16)

    in_maps = []
    for b in range(B):
        cnt = np.bincount(tok_s[b][tok_s[b] >= 0], minlength=1025).astype(np.float64)
        rcS_h = np.ones((P, 5), np.float32)
        rcT_h = np.ones((P, 5), np.float32)
        for j in range(5):
            for p in range(P):
                s_src = 128 * j + p
                if 1 <= s_src <= 512:
                    rcS_h[p, j] = 1.0 / max(cnt[s_src], 1.0)
                s_tgt = 128 * (j + 4) + p
                if 513 <= s_tgt <= 1024:
                    rcT_h[p, j] = 1.0 / max(cnt[s_tgt], 1.0)
        slo_t = slo_v[b].reshape(NT, P).T.astype(np.float32)  # [128, NT]
        slo_ct = np.full((P, max(NCT, 1)), -1.0, np.float32)
        for k, (i, uv) in enumerate(ct_cols):
            m = u[b, i * P:(i + 1) * P] == uv
            slo_ct[m, k] = slo_t[m, i]
        qr = np.arange(P, dtype=np.float32)
        cls_h = (np.concatenate([slo_t, slo_ct], axis=1)[:, :, None]
                 == qr[None, None, :]).astype(ml_dtypes.bfloat16)
        ch_h = np.zeros((P, CW), np.float32)
        for k, (i, cname, uv) in enumerate(ch_cols):
            m = (is_t if cname == "tgt" else is_s)[b, i * P:(i + 1) * P]
            ch_h[:, k] = (m & (u[b, i * P:(i + 1) * P] == uv)).astype(np.float32)
        biascol = np.full((P, 1), bias, np.float32)
        cf_h = np.concatenate([ch_h, rcS_h, rcT_h, biascol], axis=1)
        cb_h = np.concatenate(
            [s1_h, s2_h, ident_h, iota_h, np.zeros((P, 12), np.float32)],
            axis=1).astype(ml_dtypes.bfloat16)
        in_maps.append({
            "x": xbs[b],
            "consts": np.ascontiguousarray(cf_h.astype(np.float32)),
            "cbf": np.ascontiguousarray(cb_h),
            "clsb": np.ascontiguousarray(cls_h.reshape(P, -1)),
            "wrepb": np.ascontiguousarray(wrep_h),
        })
    ct = dict(start=ct_start, plan=ct_plan, first_u=first_u, last_u=last_u, NCT=NCT)
    return NT, ops, CW, lt_tgt, lt_s1, ct, in_maps


def _run(inputs, trace=False, tmpdir=None):
    NT, ops, CW, lt_tgt, lt_s1, ct, in_maps = _host_prep(inputs)
    nc = _build_nc(NT, ops, CW, lt_tgt, lt_s1, ct)
    res = run_bass_kernel_spmd(nc, in_maps, core_ids=list(range(8)), trace=trace, tmpdir=tmpdir)
    out = np.stack([np.asarray(r["y"]).astype(np.float32) for r in res.results])
    return out, res


def kernel(**inputs) -> np.ndarray:
    out, _ = _run(inputs, trace=False)
    return out


if __name__ == "__main__":
    # CoreSim smoke test on core 0's inputs
    import jax
    jax.config.update("jax_platforms", "cpu")
    sys.path.insert(0, "/root/problem")
    import reference as ref
    from concourse.bass_interp import CoreSim

    inputs = ref.setup_inputs()
    NT, ops, CW, lt_tgt, lt_s1, ct, in_maps = _host_prep(inputs)
    print("NT =", NT, "CW =", CW, "lt_tgt =", lt_tgt, "lt_s1 =", lt_s1, "ct =", ct)
    for i, ent in enumerate(ops):
        print(i, [(e["c"], e["ulist"], e["red"]) for e in ent])
    nc = _build_nc(NT, ops, CW, lt_tgt, lt_s1, ct)
    sim = CoreSim(nc)
    for name, arr in in_maps[0].items():
        sim.tensor(name)[:] = arr
    sim.simulate()
    got = np.array(sim.tensor("y")).astype(np.float32)
    expected = np.asarray(ref.reference(**inputs))[0]
    err = np.abs(got - expected).max()
    scale = np.abs(expected).max()
    print("CoreSim abs err:", err, "rel:", err / scale)
    assert err / scale < 1e-2, "CoreSim mismatch"
    print("CORESIM PASSES")

